# revision 1
# baseline (speedup 1.0000x reference)
"""Causal self-attention TP kernel for 8 trn2 NeuronCores.

Problem shapes (hardcoded): x [2, 2048, 2048] f32, w_attn [2048, 6144],
w_proj [2048, 2048], 16 heads, head_dim 128.

Sharding: tensor-parallel over heads — core i owns heads {2i, 2i+1} for BOTH
batches. Each core computes its local-head qkv + attention, producing
y_local^T [512 feat, 4096 tok]. Two 8-core AllToAlls (one per batch)
re-shard from feature-split to token-split: core g receives
y^T[all 2048 feat, 256 tokens of each batch] and projects those 512 tokens
against the full w_proj, emitting out[512, 2048] (batch0 rows then batch1).
The batch-0 AllToAll overlaps batch-1 attention compute; the final
projection overlaps the batch-1 AllToAll.

All matmuls run as float32r (FP22, full PE rate at free-dim >= 256); the
BIR verifier requires fp32r matmul operands to be *produced* with fp32r
dtype, so operand tiles are allocated as F32R and producers write
f32r-labeled APs (identical 4-byte bits; the PE truncates on read).

Softmax: no max-subtraction (scores ~N(0,1), exp is safe in fp32); row
sums via a per-tile ones-matmul on the PE (partition-axis reduction);
normalization applied to y after the PV matmul via a DMA-broadcast
reciprocal.
"""

import os
import numpy as np

import concourse.bass as bass
import concourse.mybir as mybir
import concourse.tile as tile
from concourse import bacc
from concourse.bass_utils import run_bass_kernel_spmd

F32 = mybir.dt.float32
F32R = mybir.dt.float32r

B, T, C = 2, 2048, 2048
H, D = 16, 128
NTOK = B * T                     # 4096 flat tokens (batch-major)
SCALE = 1.0 / float(np.sqrt(D))  # 0.08838834764831845
NCORES = 8
HPC = H // NCORES                # 2 heads per core
FLOC = HPC * D                   # 256 local v features
QK = 512                         # q+k local features (2 heads x 128 x 2)

last_exec_time_ns = None
_cache = {}


def r32(ap):
    return ap.bitcast(F32R)


def _masks_np():
    # mask[m, kk, qq] = 1.0 iff kk <= qq - 128*m   (for diagonal tile offset m)
    m = np.arange(4)[:, None, None]
    kk = np.arange(128)[None, :, None]
    qq = np.arange(512)[None, None, :]
    return (kk <= qq - 128 * m).astype(np.float32)


def build_nc(no_collective=False, reps=1):
    nc = bacc.Bacc("TRN2", target_bir_lowering=False, debug=False,
                   num_devices=1 if no_collective else NCORES)

    xt = nc.dram_tensor("xt", [C, NTOK], F32, kind="ExternalInput")
    wqk = nc.dram_tensor("wqk", [C, QK], F32, kind="ExternalInput")
    wv = nc.dram_tensor("wv", [C, FLOC], F32, kind="ExternalInput")
    wp = nc.dram_tensor("wp", [C, C], F32, kind="ExternalInput")
    out = nc.dram_tensor("out", [512, C], F32, kind="ExternalOutput")

    v_dram = nc.dram_tensor("v_dram", [NTOK, FLOC], F32)
    # per-batch a2a buffers: 8 shards x [256 feat x 256 tok]
    y_loc = [nc.dram_tensor(f"y_loc{b}", [2048, 256], F32) for b in range(B)]
    y_t = [nc.dram_tensor(f"y_t{b}", [2048, 256], F32) for b in range(B)]
    masks = nc.inline_tensor(_masks_np(), "masks")      # [4, 128, 512]
    ones_dr = nc.inline_tensor(np.ones((128, 1), np.float32), "ones_c")
    zeros_dr = nc.inline_tensor(np.zeros((128, 1), np.float32), "zeros_c")
    onesr_dr = nc.inline_tensor(np.ones((1, 128), np.float32), "onesr_c")

    def a2a(b):
        if no_collective:
            nc.sync.dma_start(out=y_t[b][:, :], in_=y_loc[b][:, :])
        else:
            nc.gpsimd.collective_compute(
                "AllToAll",
                mybir.AluOpType.bypass,
                replica_groups=[list(range(NCORES))],
                ins=[y_loc[b][:, :]],
                outs=[y_t[b][:, :]],
            )

    with tile.TileContext(nc) as tc:
      for _rep in range(reps):
        # ---- persistent (qkv outputs + constants), freed after attention ----
        with tc.tile_pool(name="persist", bufs=1) as persist:
            # q^T,k^T for 2 heads, all tokens: chunk f = {q_h0, q_h1, k_h0, k_h1}
            qk_res = persist.tile([128, 4, NTOK], F32R)
            ones_sb = persist.tile([128, 1], F32R)
            zeros_sb = persist.tile([128, 1], F32)
            nc.gpsimd.dma_start(out=zeros_sb, in_=zeros_dr.ap())
            onesr_sb = persist.tile([1, 128], F32R)
            nc.gpsimd.dma_start(out=onesr_sb, in_=r32(onesr_dr.ap()))
            scr = persist.tile([128, 1], F32)
            # warm the ACT exp table set (~2.7us) before attention needs it
            nc.scalar.activation(scr, ones_sb.bitcast(F32),
                                 mybir.ActivationFunctionType.Exp, bias=zeros_sb)
            # constants go through the gpsimd DMA queue to keep the sync-engine
            # queue free for the latency-critical weight/x loads at startup
            nc.gpsimd.dma_start(out=ones_sb, in_=r32(ones_dr.ap()))

            # ================= phase 1: qkv =================
            p2v_ctx = tc.tile_pool(name="p2v", bufs=3)
            p2v = p2v_ctx.__enter__()
            v_pre = {}

            def load_v(b, h):
                v_sb = p2v.tile([128, 16, 128], F32R, tag="vsb",
                                name=f"v_sb{b}{h}")
                for vc in range(16):
                    nc.sync.dma_start(
                        out=v_sb[:, vc, :],
                        in_=r32(v_dram[b * T + vc * 128: b * T + (vc + 1) * 128,
                                       h * 128:(h + 1) * 128]),
                    )
                return v_sb

            with (
                tc.tile_pool(name="p1w", bufs=1) as p1w,
                tc.tile_pool(name="p1x", bufs=3) as p1x,
                tc.tile_pool(name="p1s", bufs=4) as p1s,
                tc.tile_pool(name="p1ps", bufs=4, space="PSUM") as p1ps,
                tc.tile_pool(name="p1psv", bufs=2, space="PSUM") as p1psv,
            ):
                wqk_sb = p1w.tile([128, 16, QK], F32R)
                wv_sb = p1w.tile([128, 16, FLOC], F32R)

                for tt in range(8):          # 512-token tiles over 4096 flat tokens
                    xh = []
                    for half in range(2):    # 8 c-chunks per half
                        xbuf = p1x.tile([128, 8, 512], F32R, tag="xh")
                        c0 = half * 8
                        for cc in range(8):
                            if tt == 0:
                                # interleave weight-chunk loads with the first
                                # x-tile loads so the c-loop matmuls can chase
                                # the DMA stream from ~2us in
                                nc.sync.dma_start(
                                    out=wqk_sb[:, c0 + cc, :],
                                    in_=r32(wqk[(c0 + cc) * 128:(c0 + cc + 1) * 128, :]))
                            nc.sync.dma_start(
                                out=xbuf[:, cc, :],
                                in_=r32(xt[(c0 + cc) * 128:(c0 + cc + 1) * 128,
                                           tt * 512:(tt + 1) * 512]),
                            )
                        xh.append(xbuf)
                    if tt == 0:
                        # wv is first needed ~14us in; load it after tt0's x
                        for c in range(16):
                            nc.sync.dma_start(
                                out=wv_sb[:, c, :],
                                in_=r32(wv[c * 128:(c + 1) * 128, :]))
                    # q^T / k^T feature blocks
                    for fb in range(4):
                        ps = p1ps.tile([128, 512], F32, tag="qkps")
                        for half in range(2):
                            for cc in range(8):
                                c = half * 8 + cc
                                nc.tensor.matmul(
                                    ps,
                                    lhsT=wqk_sb[:, c, fb * 128:(fb + 1) * 128],
                                    rhs=xh[half][:, cc, :],
                                    start=(c == 0), stop=(c == 15),
                                )
                        nc.vector.tensor_copy(qk_res[:, fb, tt * 512:(tt + 1) * 512], ps)
                    # v token blocks (token-major out)
                    for tb in range(4):
                        psv = p1psv.tile([128, FLOC], F32, tag="vps")
                        for half in range(2):
                            for cc in range(8):
                                c = half * 8 + cc
                                nc.tensor.matmul(
                                    psv,
                                    lhsT=xh[half][:, cc, tb * 128:(tb + 1) * 128],
                                    rhs=wv_sb[:, c, :],
                                    start=(c == 0), stop=(c == 15),
                                )
                        if tt < 4:
                            # batch-0 v goes straight to its attention tiles
                            if tt == 0 and tb == 0:
                                for h in range(HPC):
                                    v_pre[(0, h)] = p2v.tile(
                                        [128, 16, 128], F32R, tag="vsb",
                                        name=f"v_pre0{h}")
                            for h in range(HPC):
                                nc.vector.tensor_copy(
                                    v_pre[(0, h)][:, tt * 4 + tb, :],
                                    psv[:, h * 128:(h + 1) * 128])
                        else:
                            st = p1s.tile([128, FLOC], F32, tag="vst")
                            nc.vector.tensor_copy(st, psv)
                            nc.sync.dma_start(
                                out=v_dram[tt * 512 + tb * 128:
                                           tt * 512 + (tb + 1) * 128, :],
                                in_=st,
                            )

            # ============ phases 2+3: attention + per-batch a2a ============
            # proj pools open early so the first w_proj slice prefetches
            # during attention (their SBUF must not overlap phase-1 pools)
            with (
                tc.tile_pool(name="p4w", bufs=3) as p4w,
                tc.tile_pool(name="p4y", bufs=1) as p4y,
                tc.tile_pool(name="p4s", bufs=4) as p4s,
                tc.tile_pool(name="p4ps", bufs=1, space="PSUM") as p4ps,
            ):
                _wpn = [0]

                def load_wp_chunk(ch):
                    wt = p4w.tile([128, 16, 256], F32R, tag="wp",
                                  name=f"wp_t{_wpn[0]}")
                    _wpn[0] += 1
                    nc.sync.dma_start(
                        out=wt,
                        in_=r32(wp[:, ch * 256:(ch + 1) * 256].rearrange(
                            "(n p) f -> p n f", p=128)))
                    return wt

                def load_yt(b):
                    yb = p4y.tile([128, 16, 256], F32R, tag=f"yt{b}", name=f"yt{b}")
                    nc.sync.dma_start(
                        out=yb,
                        in_=r32(y_t[b].ap().rearrange("(n p) t -> p n t", p=128)))
                    return yb

                def proj_all(yts, wp_tiles):
                    """combined projection: one w_proj sweep. Batch-0 groups on
                    the prefetched chunks run first (they don't wait on the
                    batch-1 all-to-all); batch-1 + remaining chunks follow."""
                    npre = len(wp_tiles)
                    order = [(ch, 0) for ch in range(npre)]
                    order += [(ch, 1) for ch in range(npre)]
                    for ch in range(npre, 8):
                        order += [(ch, 0), (ch, 1)]
                    for ch, b in order:
                        if ch < npre:
                            wt = wp_tiles[ch]
                            # warm the stream: issue the next unloaded chunk's
                            # DMA one step ahead of its consuming groups
                            if b == 1 and ch + npre < 8 and len(wp_tiles) < 8:
                                wp_tiles.append(load_wp_chunk(len(wp_tiles)))
                        else:
                            while len(wp_tiles) <= min(ch + 1, 7):
                                wp_tiles.append(load_wp_chunk(len(wp_tiles)))
                            wt = wp_tiles[ch]
                        if True:
                            for tb in range(2):
                                ps = p4ps.tile([128, 256], F32, tag="ops")
                                for c in range(16):
                                    nc.tensor.matmul(
                                        ps,
                                        lhsT=yts[b][:, c, tb * 128:(tb + 1) * 128],
                                        rhs=wt[:, c, :],
                                        start=(c == 0), stop=(c == 15),
                                    )
                                st = p4s.tile([128, 256], F32, tag="ost")
                                nc.vector.tensor_copy(st, ps)
                                nc.sync.dma_start(
                                    out=out[b * 256 + tb * 128: b * 256 + (tb + 1) * 128,
                                            ch * 256:(ch + 1) * 256],
                                    in_=st,
                                )

                with (
                    tc.tile_pool(name="p2m", bufs=1) as p2m,
                    tc.tile_pool(name="p2p", bufs=5) as p2p,
                    tc.tile_pool(name="p2y", bufs=2) as p2y,
                    tc.tile_pool(name="p2r", bufs=3) as p2r,
                    tc.tile_pool(name="p2rd", bufs=2, space="DRAM") as p2rd,
                    tc.tile_pool(name="p2pss", bufs=3, space="PSUM") as p2pss,
                    tc.tile_pool(name="p2psy", bufs=2, space="PSUM") as p2psy,
                    tc.tile_pool(name="p2psr", bufs=2, space="PSUM") as p2psr,
                ):
                    mask_sb = p2m.tile([128, 4, 512], F32)
                    nc.sync.dma_start(out=mask_sb,
                                       in_=masks.ap().rearrange("m p q -> p m q"))
                    wp_pending = []   # w_proj chunks prefetched in b1 window
                    yts = []
                    for b in range(B):
                        nhj = 0
                        for h in range(HPC):
                            v_sb = v_pre.pop((b, h), None) or load_v(b, h)
                            qf, kf = h, 2 + h
                            tok0 = b * T
                            for j in range(4):
                                if b == 1 and nhj < 3:
                                    # stream next-proj w_proj chunk loads through
                                    # the sync FIFO during batch-1 attention
                                    wp_pending.append(load_wp_chunk(nhj))
                                    nhj += 1
                                nk = 4 * j + 4
                                y_ps = p2psy.tile([128, 512], F32, tag="yps")
                                r_ps = p2psr.tile([1, 512], F32, tag="rps")
                                qs = qk_res[:, qf, tok0 + j * 512: tok0 + (j + 1) * 512]
                                for c in range(nk):
                                    s_ps = p2pss.tile([128, 512], F32, tag="sps")
                                    nc.tensor.matmul(
                                        s_ps,
                                        lhsT=qk_res[:, kf,
                                                    tok0 + c * 128: tok0 + (c + 1) * 128],
                                        rhs=qs,
                                        start=True, stop=True,
                                    )
                                    p_sb = p2p.tile([128, 512], F32R, tag="p")
                                    nc.scalar.activation(
                                        p_sb, s_ps,
                                        mybir.ActivationFunctionType.Exp,
                                        scale=SCALE, bias=zeros_sb,
                                    )
                                    if c >= 4 * j:
                                        nc.vector.tensor_mul(
                                            p_sb, p_sb, mask_sb[:, c - 4 * j, :])
                                    nc.tensor.matmul(
                                        y_ps,
                                        lhsT=v_sb[:, c, :],
                                        rhs=p_sb,
                                        start=(c == 0), stop=(c == nk - 1),
                                    )
                                    nc.tensor.matmul(
                                        r_ps,
                                        lhsT=ones_sb,
                                        rhs=p_sb,
                                        start=(c == 0), stop=(c == nk - 1),
                                    )
                                rr = p2r.tile([1, 512], F32, tag="rr")
                                nc.vector.reciprocal(rr, r_ps)
                                rb = p2r.tile([128, 512], F32, tag="rb")
                                nc.gpsimd.partition_broadcast(rb, rr)
                                y_sb = p2y.tile([128, 512], F32, tag="ysb")
                                nc.vector.tensor_mul(y_sb, y_ps, rb)
                                # token eighths 2j, 2j+1 of batch b
                                for e in range(2):
                                    s = 2 * j + e
                                    nc.sync.dma_start(
                                        out=y_loc[b][s * 256 + h * 128:
                                                     s * 256 + (h + 1) * 128, :],
                                        in_=y_sb[:, e * 256:(e + 1) * 256],
                                    )
                        # batch-b all-to-all; b=0's overlaps b=1 attention
                        a2a(b)
                        yts.append(load_yt(b))
                    proj_all(yts, wp_pending)
            p2v_ctx.__exit__(None, None, None)

    nc.compile()
    return nc


def kernel(x, w_attn, w_proj):
    global last_exec_time_ns
    x = np.asarray(x, dtype=np.float32)
    w_attn = np.asarray(w_attn, dtype=np.float32)
    w_proj = np.asarray(w_proj, dtype=np.float32)

    if "nc" not in _cache:
        _cache["nc"] = build_nc()
    nc = _cache["nc"]

    xt = np.ascontiguousarray(x.reshape(NTOK, C).T)          # [C, NTOK]
    wp = np.ascontiguousarray(w_proj)
    in_maps = []
    for i in range(NCORES):
        qcols = w_attn[:, FLOC * i: FLOC * (i + 1)]
        kcols = w_attn[:, C + FLOC * i: C + FLOC * (i + 1)]
        vcols = w_attn[:, 2 * C + FLOC * i: 2 * C + FLOC * (i + 1)]
        in_maps.append({
            "xt": xt,
            "wqk": np.ascontiguousarray(np.concatenate([qcols, kcols], axis=1)),
            "wv": np.ascontiguousarray(vcols),
            "wp": wp,
        })

    res = run_bass_kernel_spmd(nc, in_maps, list(range(NCORES)))
    last_exec_time_ns = res.exec_time_ns

    return assemble([res.results[g]["out"] for g in range(NCORES)])


def assemble(outs):
    # core g's out rows: [0:256] = batch0 tokens [256g:256(g+1)],
    #                    [256:512] = batch1 tokens [256g:256(g+1)]
    full = np.empty((B, T, C), np.float32)
    for g in range(NCORES):
        for b in range(B):
            full[b, 256 * g: 256 * (g + 1), :] = outs[g][b * 256:(b + 1) * 256]
    return full



# revision 11
# speedup vs baseline: 1.1152x; 1.1152x over previous
"""Causal self-attention TP kernel for 8 trn2 NeuronCores.

Problem shapes (hardcoded): x [2, 2048, 2048] f32, w_attn [2048, 6144],
w_proj [2048, 2048], 16 heads, head_dim 128.

Sharding: tensor-parallel over heads — core i owns heads {2i, 2i+1} for BOTH
batches. Each core computes its local-head qkv + attention, producing
y_local^T [256 feat, 4096 tok] (unnormalized) plus per-token softmax row
sums r. Two small AllToAlls per batch (r then y) re-shard from feature-split
to token-split: core g receives y^T[all 2048 feat, 256 tokens] + r[16 heads,
256 tokens], normalizes y by 1/r per head (commutes with the projection per
head-block), and projects its 512 tokens against the full w_proj.

Precision: q/k/x/w_attn stay fp32r (full PE rate); p = exp(scores), v, y,
w_proj are bf16 (same PE rate, 2x DVE, half the DMA/a2a bytes). Row sums
accumulate in f32. Causal mask is folded into the score PSUM accumulation
as an extra identity x (-1e9 staircase) matmul, so no DVE masking.

Row sums: most k-tiles accumulate P on the DVE into r_acc (f32); every 8th
tile goes through a per-tile ones-matmul on the PE; one final ones-matmul
per q-tile folds r_acc into the same PSUM accumulator.
"""

import numpy as np
import ml_dtypes

import concourse.bass as bass
import concourse.mybir as mybir
import concourse.tile as tile
from concourse import bacc
from concourse.bass_utils import run_bass_kernel_spmd

F32 = mybir.dt.float32
F32R = mybir.dt.float32r
BF16 = mybir.dt.bfloat16
NPBF16 = ml_dtypes.bfloat16

B, T, C = 2, 2048, 2048
H, D = 16, 128
NTOK = B * T                     # 4096 flat tokens (batch-major)
SCALE = 1.0 / float(np.sqrt(D))  # 0.08838834764831845
NCORES = 8
HPC = H // NCORES                # 2 heads per core
FLOC = HPC * D                   # 256 local v features
QK = 512                         # q+k local features (2 heads x 128 x 2)
NEG = -1.0e9                     # causal mask bias (exp(SCALE*NEG) == 0)

last_exec_time_ns = None
_cache = {}


def r32(ap):
    return ap.bitcast(F32R)


def _maskbias_np():
    # mb[m, kk, qq] = 0 iff kk <= qq - 128*m else NEG (diagonal tile offset m)
    m = np.arange(4)[:, None, None]
    kk = np.arange(128)[None, :, None]
    qq = np.arange(512)[None, None, :]
    return np.where(kk <= qq - 128 * m, 0.0, NEG).astype(np.float32)


def build_nc(no_collective=False, reps=1):
    nc = bacc.Bacc("TRN2", target_bir_lowering=False, debug=False,
                   num_devices=1 if no_collective else NCORES)

    xt = nc.dram_tensor("xt", [C, NTOK], F32, kind="ExternalInput")
    wqk = nc.dram_tensor("wqk", [C, QK], F32, kind="ExternalInput")
    wv = nc.dram_tensor("wv", [C, FLOC], F32, kind="ExternalInput")
    wp = nc.dram_tensor("wp", [C, C], BF16, kind="ExternalInput")
    out = nc.dram_tensor("out", [512, C], F32, kind="ExternalOutput")

    # per-batch a2a buffers: y (8 shards x [256 feat x 256 tok], bf16) and
    # softmax row sums r (8 shards x [2 heads x 256 tok], f32)
    y_loc = [nc.dram_tensor(f"y_loc{b}", [2048, 256], BF16) for b in range(B)]
    y_t = [nc.dram_tensor(f"y_t{b}", [2048, 256], BF16) for b in range(B)]
    r_loc = [nc.dram_tensor(f"r_loc{b}", [16, 256], F32) for b in range(B)]
    r_t = [nc.dram_tensor(f"r_t{b}", [16, 256], F32) for b in range(B)]
    ri_dram = [nc.dram_tensor(f"ri{b}", [1, 16, 256], BF16) for b in range(B)]

    maskb_dr = nc.inline_tensor(_maskbias_np(), "maskb")      # [4, 128, 512]
    ident_dr = nc.inline_tensor(np.eye(128, dtype=np.float32), "ident")
    ones_dr = nc.inline_tensor(np.ones((128, 1), np.float32), "ones_c")
    onesb_dr = nc.inline_tensor(np.ones((128, 1), NPBF16), "onesb_c")
    zeros_dr = nc.inline_tensor(np.zeros((128, 1), np.float32), "zeros_c")

    def a2a(b):
        if no_collective:
            nc.gpsimd.dma_start(out=r_t[b][:, :], in_=r_loc[b][:, :])
            nc.gpsimd.dma_start(out=y_t[b][:, :], in_=y_loc[b][:, :])
        else:
            nc.gpsimd.collective_compute(
                "AllToAll", mybir.AluOpType.bypass,
                replica_groups=[list(range(NCORES))],
                ins=[r_loc[b][:, :]], outs=[r_t[b][:, :]],
            )
            nc.gpsimd.collective_compute(
                "AllToAll", mybir.AluOpType.bypass,
                replica_groups=[list(range(NCORES))],
                ins=[y_loc[b][:, :]], outs=[y_t[b][:, :]],
            )

    with tile.TileContext(nc) as tc:
      for _rep in range(reps):
        # ---- persistent (qkv outputs + constants) ----
        with tc.tile_pool(name="persist", bufs=1) as persist:
            # q^T,k^T for 2 heads, all tokens: chunk f = {q_h0, q_h1, k_h0, k_h1}
            qk_res = persist.tile([128, 4, NTOK], F32R)
            ones_sb = persist.tile([128, 1], F32R)
            onesb_sb = persist.tile([128, 1], BF16)
            zeros_sb = persist.tile([128, 1], F32)
            nc.gpsimd.dma_start(out=zeros_sb, in_=zeros_dr.ap())
            maskb_sb = persist.tile([128, 4, 512], F32R)
            nc.gpsimd.dma_start(out=maskb_sb,
                                in_=r32(maskb_dr.ap().rearrange("m p q -> p m q")))
            ident_sb = persist.tile([128, 128], F32R)
            nc.gpsimd.dma_start(out=ident_sb, in_=r32(ident_dr.ap()))
            scr = persist.tile([128, 1], F32)
            # warm the ACT exp table set (~2.7us) before attention needs it
            nc.scalar.activation(scr, zeros_sb,
                                 mybir.ActivationFunctionType.Exp, bias=zeros_sb)
            nc.gpsimd.dma_start(out=ones_sb, in_=r32(ones_dr.ap()))
            nc.gpsimd.dma_start(out=onesb_sb, in_=onesb_dr.ap())

            # v for all (batch, head), bf16, SBUF-resident: [tok128, ktile, feat]
            p2v_ctx = tc.tile_pool(name="p2v", bufs=4)
            p2v = p2v_ctx.__enter__()
            v_pre = {}

            # ================= phase 1: qkv =================
            with (
                tc.tile_pool(name="p1w", bufs=1) as p1w,
                tc.tile_pool(name="p1x", bufs=3) as p1x,
                tc.tile_pool(name="p1ps", bufs=4, space="PSUM") as p1ps,
                tc.tile_pool(name="p1psv", bufs=4, space="PSUM") as p1psv,
            ):
                wqk_sb = p1w.tile([128, 16, QK], F32R)
                wv_sb = p1w.tile([128, 16, FLOC], F32R)

                for tt in range(8):          # 512-token tiles over 4096 tokens
                    b = tt // 4
                    xh = []
                    for half in range(2):
                        xbuf = p1x.tile([128, 8, 512], F32R, tag="xh")
                        c0 = half * 8
                        if tt == 0:
                            # chase: per-chunk loads, weights on sync queue,
                            # x on the (idle) vector queue, paired per chunk
                            for cc in range(8):
                                nc.sync.dma_start(
                                    out=wqk_sb[:, c0 + cc, :],
                                    in_=r32(wqk[(c0 + cc) * 128:(c0 + cc + 1) * 128, :]))
                                nc.scalar.dma_start(
                                    out=xbuf[:, cc, :],
                                    in_=r32(xt[(c0 + cc) * 128:(c0 + cc + 1) * 128,
                                               tt * 512:(tt + 1) * 512]))
                        else:
                            nc.sync.dma_start(
                                out=xbuf,
                                in_=r32(xt[c0 * 128:(c0 + 8) * 128,
                                           tt * 512:(tt + 1) * 512].rearrange(
                                               "(n p) f -> p n f", p=128)))
                        xh.append(xbuf)
                    if tt == 0:
                        nc.sync.dma_start(
                            out=wv_sb,
                            in_=r32(wv.ap().rearrange("(n p) f -> p n f", p=128)))
                    if tt % 4 == 0:
                        for h in range(HPC):
                            v_pre[(b, h)] = p2v.tile(
                                [128, 16, 128], BF16, tag="vsb", name=f"v{b}{h}")
                    # c-outer so compute chases the DMA stream chunk by chunk
                    ps = [p1ps.tile([128, 512], F32, tag="qkps", name=f"qk{fb}")
                          for fb in range(4)]
                    psv = [p1psv.tile([128, FLOC], F32, tag="vps", name=f"v{tb}")
                           for tb in range(4)]
                    for c in range(16):
                        half, cc = c // 8, c % 8
                        for fb in range(4):
                            nc.tensor.matmul(
                                ps[fb],
                                lhsT=wqk_sb[:, c, fb * 128:(fb + 1) * 128],
                                rhs=xh[half][:, cc, :],
                                start=(c == 0), stop=(c == 15),
                            )
                        for tb in range(4):
                            nc.tensor.matmul(
                                psv[tb],
                                lhsT=xh[half][:, cc, tb * 128:(tb + 1) * 128],
                                rhs=wv_sb[:, c, :],
                                start=(c == 0), stop=(c == 15),
                            )
                    for fb in range(4):
                        nc.vector.tensor_copy(
                            qk_res[:, fb, tt * 512:(tt + 1) * 512], ps[fb])
                    for tb in range(4):
                        for h in range(HPC):
                            nc.vector.tensor_copy(
                                v_pre[(b, h)][:, (tt % 4) * 4 + tb, :],
                                psv[tb][:, h * 128:(h + 1) * 128])

            # ============ phases 2+3: attention + per-batch a2a + proj ======
            with (
                tc.tile_pool(name="p4w", bufs=4) as p4w,
                tc.tile_pool(name="p4y", bufs=2) as p4y,
                tc.tile_pool(name="p4r", bufs=2) as p4r,
                tc.tile_pool(name="p4rb", bufs=1) as p4rb,
                tc.tile_pool(name="p4s", bufs=3) as p4s,
                tc.tile_pool(name="p4ps", bufs=2, space="PSUM") as p4ps,
            ):
                wp_tiles = []

                def load_wp_chunk(ch):
                    wt = p4w.tile([128, 16, 512], BF16, tag="wp", name=f"wp{ch}")
                    nc.sync.dma_start(
                        out=wt,
                        in_=wp[:, ch * 512:(ch + 1) * 512].rearrange(
                            "(n p) f -> p n f", p=128))
                    wp_tiles.append(wt)

                yts, rbs = [], []

                def recv_batch(b):
                    """post-a2a: load y/r for my 512 tokens, normalize y."""
                    rr = p4r.tile([16, 256], F32, tag="rrecv", name=f"rr{b}")
                    nc.sync.dma_start(out=rr, in_=r_t[b].ap())
                    ri = p4r.tile([16, 256], BF16, tag="rinv", name=f"ri{b}")
                    with nc.allow_low_precision(reason="softmax denom bf16"):
                        nc.vector.reciprocal(ri, rr)
                    nc.gpsimd.dma_start(out=ri_dram[b].ap(), in_=ri)
                    rb = p4rb.tile([128, 16, 256], BF16, tag="rb", name=f"rb{b}")
                    nc.gpsimd.dma_start(
                        out=rb, in_=ri_dram[b].ap().to_broadcast([128, 16, 256]))
                    yb = p4y.tile([128, 16, 256], BF16, tag="yt", name=f"yt{b}")
                    nc.sync.dma_start(
                        out=yb,
                        in_=y_t[b].ap().rearrange("(n p) t -> p n t", p=128))
                    nc.vector.tensor_mul(yb, yb, rb)
                    yts.append(yb)
                    rbs.append(rb)

                with (
                    tc.tile_pool(name="p2p", bufs=3) as p2p,
                    tc.tile_pool(name="p2r", bufs=2) as p2r,
                    tc.tile_pool(name="p2y", bufs=2) as p2y,
                    tc.tile_pool(name="p2pss", bufs=2, space="PSUM") as p2pss,
                    tc.tile_pool(name="p2psy", bufs=1, space="PSUM") as p2psy,
                    tc.tile_pool(name="p2psr", bufs=1, space="PSUM") as p2psr,
                ):
                    nwp = [0]
                    for b in range(B):
                        for h in range(HPC):
                            v_sb = v_pre[(b, h)]
                            qf, kf = h, 2 + h
                            tok0 = b * T
                            for j in range(4):
                                if b == 1 and h == 0 and nwp[0] < 3:
                                    # stream w_proj chunks during b1 attention
                                    load_wp_chunk(nwp[0])
                                    nwp[0] += 1
                                nk = 4 * j + 4
                                y_ps = p2psy.tile([128, 512], F32, tag="yps")
                                r_ps = p2psr.tile([1, 512], F32, tag="rps")
                                r_acc = p2r.tile([128, 512], F32R, tag="racc")
                                qs = qk_res[:, qf,
                                            tok0 + j * 512: tok0 + (j + 1) * 512]
                                pe_rows = [c for c in range(nk) if c % 8 == 7]
                                ndve = 0
                                for t in range(nk // 2):
                                    s_ps = p2pss.tile([128, 2, 512], F32, tag="sps")
                                    for e in range(2):
                                        c = 2 * t + e
                                        diag = c - 4 * j
                                        nc.tensor.matmul(
                                            s_ps[:, e, :],
                                            lhsT=qk_res[:, kf,
                                                        tok0 + c * 128:
                                                        tok0 + (c + 1) * 128],
                                            rhs=qs,
                                            start=True, stop=(diag < 0),
                                        )
                                        if diag >= 0:
                                            nc.tensor.matmul(
                                                s_ps[:, e, :],
                                                lhsT=ident_sb,
                                                rhs=maskb_sb[:, diag, :],
                                                start=False, stop=True,
                                            )
                                    p_sb = p2p.tile([128, 2, 512], BF16, tag="p")
                                    nc.scalar.activation(
                                        p_sb, s_ps,
                                        mybir.ActivationFunctionType.Exp,
                                        scale=SCALE, bias=zeros_sb,
                                    )
                                    for e in range(2):
                                        c = 2 * t + e
                                        nc.tensor.matmul(
                                            y_ps,
                                            lhsT=v_sb[:, c, :],
                                            rhs=p_sb[:, e, :],
                                            start=(c == 0), stop=(c == nk - 1),
                                        )
                                        if c in pe_rows:
                                            nc.tensor.matmul(
                                                r_ps,
                                                lhsT=onesb_sb,
                                                rhs=p_sb[:, e, :],
                                                start=(c == pe_rows[0]),
                                                stop=False,
                                            )
                                        elif ndve == 0:
                                            nc.vector.tensor_copy(
                                                r_acc, p_sb[:, e, :])
                                            ndve += 1
                                        else:
                                            nc.vector.tensor_add(
                                                r_acc, r_acc, p_sb[:, e, :])
                                            ndve += 1
                                # fold the DVE-accumulated part into r_ps
                                nc.tensor.matmul(
                                    r_ps, lhsT=ones_sb, rhs=r_acc,
                                    start=(len(pe_rows) == 0), stop=True,
                                )
                                # r rows -> r_loc (shard s=2j+e, head h)
                                r_sb = p2r.tile([1, 512], F32, tag="rsb")
                                nc.vector.tensor_copy(r_sb, r_ps)
                                for e in range(2):
                                    s = 2 * j + e
                                    nc.gpsimd.dma_start(
                                        out=r_loc[b][2 * s + h:2 * s + h + 1, :],
                                        in_=r_sb[0:1, e * 256:(e + 1) * 256],
                                    )
                                y_sb = p2y.tile([128, 512], BF16, tag="ysb")
                                nc.vector.tensor_copy(y_sb, y_ps)
                                for e in range(2):
                                    s = 2 * j + e
                                    nc.gpsimd.dma_start(
                                        out=y_loc[b][s * 256 + h * 128:
                                                     s * 256 + (h + 1) * 128, :],
                                        in_=y_sb[:, e * 256:(e + 1) * 256],
                                    )
                        # batch-b a2a (r then y); b=0's overlaps b=1 attention
                        a2a(b)
                        recv_batch(b)
                    while nwp[0] < 4:
                        load_wp_chunk(nwp[0])
                        nwp[0] += 1
                    # projection: all of b0 (overlaps b1 a2a tail), then b1
                    for b in range(B):
                        for ch in range(4):
                            for tb in range(2):
                                pps = p4ps.tile([128, 512], F32, tag="ops")
                                for c in range(16):
                                    nc.tensor.matmul(
                                        pps,
                                        lhsT=yts[b][:, c, tb * 128:(tb + 1) * 128],
                                        rhs=wp_tiles[ch][:, c, :],
                                        start=(c == 0), stop=(c == 15),
                                    )
                                st = p4s.tile([128, 512], F32, tag="ost")
                                nc.vector.tensor_copy(st, pps)
                                nc.sync.dma_start(
                                    out=out[b * 256 + tb * 128:
                                            b * 256 + (tb + 1) * 128,
                                            ch * 512:(ch + 1) * 512],
                                    in_=st,
                                )
            p2v_ctx.__exit__(None, None, None)

    nc.compile()
    return nc


def make_in_maps(x, w_attn, w_proj):
    x = np.asarray(x, dtype=np.float32)
    w_attn = np.asarray(w_attn, dtype=np.float32)
    w_proj = np.asarray(w_proj, dtype=np.float32)
    xt = np.ascontiguousarray(x.reshape(NTOK, C).T)          # [C, NTOK]
    wp = np.ascontiguousarray(w_proj.astype(NPBF16))
    in_maps = []
    for i in range(NCORES):
        qcols = w_attn[:, FLOC * i: FLOC * (i + 1)]
        kcols = w_attn[:, C + FLOC * i: C + FLOC * (i + 1)]
        vcols = w_attn[:, 2 * C + FLOC * i: 2 * C + FLOC * (i + 1)]
        in_maps.append({
            "xt": xt,
            "wqk": np.ascontiguousarray(np.concatenate([qcols, kcols], axis=1)),
            "wv": np.ascontiguousarray(vcols),
            "wp": wp,
        })
    return in_maps


def kernel(x, w_attn, w_proj):
    global last_exec_time_ns
    if "nc" not in _cache:
        _cache["nc"] = build_nc()
    nc = _cache["nc"]
    in_maps = make_in_maps(x, w_attn, w_proj)
    res = run_bass_kernel_spmd(nc, in_maps, list(range(NCORES)))
    last_exec_time_ns = res.exec_time_ns
    return assemble([res.results[g]["out"] for g in range(NCORES)])


def assemble(outs):
    # core g's out rows: [0:256] = batch0 tokens [256g:256(g+1)],
    #                    [256:512] = batch1 tokens [256g:256(g+1)]
    full = np.empty((B, T, C), np.float32)
    for g in range(NCORES):
        for b in range(B):
            full[b, 256 * g: 256 * (g + 1), :] = outs[g][b * 256:(b + 1) * 256]
    return full


# revision 13
# speedup vs baseline: 1.1552x; 1.0359x over previous
"""Causal self-attention TP kernel for 8 trn2 NeuronCores.

Problem shapes (hardcoded): x [2, 2048, 2048] f32, w_attn [2048, 6144],
w_proj [2048, 2048], 16 heads, head_dim 128.

Sharding: tensor-parallel over heads — core i owns heads {2i, 2i+1} for BOTH
batches. Each core computes its local-head qkv + attention, producing
y_local^T [256 feat, 4096 tok] (unnormalized) plus per-token softmax row
sums r, shipped IN-BAND with y: one AllToAll per batch re-shards 258-row
shards [128 h0-feat | 128 h1-feat | r_h0 | r_h1] x 256 tok from
feature-split to token-split. The receiver normalizes y by 1/r per head
(normalization commutes with the projection within each head block) and
projects its 512 tokens against the full w_proj.

Precision: q/k/x/w_attn stay fp32r (full PE rate); p = exp(scores), v, y, r
(transport), w_proj are bf16; row sums accumulate in f32. The causal mask is
folded into the score PSUM accumulation as an identity x (-1e9 staircase)
matmul. Row sums: P tiles accumulate on DVE/GpSimd (alternating c, separate
f32 accumulators); every 8th tile goes through a per-tile ones-matmul on the
PE; two final ones-matmuls per q-tile fold the accumulators into PSUM.
"""

import numpy as np
import ml_dtypes

import concourse.bass as bass
import concourse.mybir as mybir
import concourse.tile as tile
from concourse import bacc
from concourse.bass_utils import run_bass_kernel_spmd

F32 = mybir.dt.float32
F32R = mybir.dt.float32r
BF16 = mybir.dt.bfloat16
NPBF16 = ml_dtypes.bfloat16

B, T, C = 2, 2048, 2048
H, D = 16, 128
NTOK = B * T                     # 4096 flat tokens (batch-major)
SCALE = 1.0 / float(np.sqrt(D))  # 0.08838834764831845
NCORES = 8
HPC = H // NCORES                # 2 heads per core
FLOC = HPC * D                   # 256 local v features
QK = 512                         # q+k local features (2 heads x 128 x 2)
NEG = -1.0e9                     # causal mask bias (exp(SCALE*NEG) == 0)
SROW = 258                       # a2a shard rows: 2x128 feat + 2 r rows

last_exec_time_ns = None
_cache = {}


def r32(ap):
    return ap.bitcast(F32R)


def _maskbias_np():
    # mb[m, kk, qq] = 0 iff kk <= qq - 128*m else NEG (diagonal tile offset m)
    m = np.arange(4)[:, None, None]
    kk = np.arange(128)[None, :, None]
    qq = np.arange(512)[None, None, :]
    return np.where(kk <= qq - 128 * m, 0.0, NEG).astype(np.float32)


def build_nc(no_collective=False, reps=1):
    nc = bacc.Bacc("TRN2", target_bir_lowering=False, debug=False,
                   num_devices=1 if no_collective else NCORES)

    xt = nc.dram_tensor("xt", [C, NTOK], F32, kind="ExternalInput")
    wqk = nc.dram_tensor("wqk", [C, QK], F32, kind="ExternalInput")
    wv = nc.dram_tensor("wv", [C, FLOC], F32, kind="ExternalInput")
    wp = nc.dram_tensor("wp", [C, C], BF16, kind="ExternalInput")
    out = nc.dram_tensor("out", [512, C], F32, kind="ExternalOutput")

    # per-batch a2a buffers: 8 shards x [258 rows x 256 tok] bf16
    y_loc = [nc.dram_tensor(f"y_loc{b}", [8 * SROW, 256], BF16) for b in range(B)]
    y_t = [nc.dram_tensor(f"y_t{b}", [8 * SROW, 256], BF16) for b in range(B)]
    ri_dram = [nc.dram_tensor(f"ri{b}", [1, 16, 256], BF16) for b in range(B)]
    warm = nc.dram_tensor("warm", [8, 2], F32)
    warm_t = nc.dram_tensor("warm_t", [8, 2], F32)

    maskb_dr = nc.inline_tensor(_maskbias_np(), "maskb")      # [4, 128, 512]
    ident_dr = nc.inline_tensor(np.eye(128, dtype=np.float32), "ident")
    ones_dr = nc.inline_tensor(np.ones((128, 1), np.float32), "ones_c")
    onesb_dr = nc.inline_tensor(np.ones((128, 1), NPBF16), "onesb_c")
    zeros_dr = nc.inline_tensor(np.zeros((128, 1), np.float32), "zeros_c")

    def a2a_op(src, dst):
        if no_collective:
            nc.gpsimd.dma_start(out=dst[:, :], in_=src[:, :])
        else:
            nc.gpsimd.collective_compute(
                "AllToAll", mybir.AluOpType.bypass,
                replica_groups=[list(range(NCORES))],
                ins=[src[:, :]], outs=[dst[:, :]],
            )

    with tile.TileContext(nc) as tc:
      for _rep in range(reps):
        # ---- persistent (qkv outputs + constants) ----
        with tc.tile_pool(name="persist", bufs=1) as persist:
            # q^T,k^T for 2 heads, all tokens: chunk f = {q_h0, q_h1, k_h0, k_h1}
            qk_res = persist.tile([128, 4, NTOK], F32R)
            ones_sb = persist.tile([128, 1], F32R)
            onesb_sb = persist.tile([128, 1], BF16)
            zeros_sb = persist.tile([128, 1], F32)
            nc.gpsimd.dma_start(out=zeros_sb, in_=zeros_dr.ap())
            maskb_sb = persist.tile([128, 4, 512], F32R)
            nc.gpsimd.dma_start(out=maskb_sb,
                                in_=r32(maskb_dr.ap().rearrange("m p q -> p m q")))
            ident_sb = persist.tile([128, 128], F32R)
            nc.gpsimd.dma_start(out=ident_sb, in_=r32(ident_dr.ap()))
            scr = persist.tile([128, 1], F32)
            # warm the ACT exp table set (~2.7us) before attention needs it
            nc.scalar.activation(scr, zeros_sb,
                                 mybir.ActivationFunctionType.Exp, bias=zeros_sb)
            nc.gpsimd.dma_start(out=ones_sb, in_=r32(ones_dr.ap()))
            nc.gpsimd.dma_start(out=onesb_sb, in_=onesb_dr.ap())
            # warm the collective stream with a dummy a2a (overlaps qkv)
            a2a_op(warm, warm_t)

            # v for all (batch, head), bf16, SBUF-resident: [tok128, ktile, feat]
            p2v_ctx = tc.tile_pool(name="p2v", bufs=4)
            p2v = p2v_ctx.__enter__()
            v_pre = {}

            # ================= phase 1: qkv =================
            with (
                tc.tile_pool(name="p1w", bufs=1) as p1w,
                tc.tile_pool(name="p1x", bufs=3) as p1x,
                tc.tile_pool(name="p1ps", bufs=4, space="PSUM") as p1ps,
                tc.tile_pool(name="p1psv", bufs=4, space="PSUM") as p1psv,
            ):
                wqk_sb = p1w.tile([128, 16, QK], F32R)
                wv_sb = p1w.tile([128, 16, FLOC], F32R)

                for tt in range(8):          # 512-token tiles over 4096 tokens
                    b = tt // 4
                    xh = []
                    for half in range(2):
                        xbuf = p1x.tile([128, 8, 512], F32R, tag="xh")
                        c0 = half * 8
                        if tt == 0:
                            # chase: per-chunk loads, weights on sync queue,
                            # x on the (idle) scalar queue, paired per chunk
                            for cc in range(8):
                                nc.sync.dma_start(
                                    out=wqk_sb[:, c0 + cc, :],
                                    in_=r32(wqk[(c0 + cc) * 128:(c0 + cc + 1) * 128, :]))
                                nc.scalar.dma_start(
                                    out=xbuf[:, cc, :],
                                    in_=r32(xt[(c0 + cc) * 128:(c0 + cc + 1) * 128,
                                               tt * 512:(tt + 1) * 512]))
                        else:
                            nc.sync.dma_start(
                                out=xbuf,
                                in_=r32(xt[c0 * 128:(c0 + 8) * 128,
                                           tt * 512:(tt + 1) * 512].rearrange(
                                               "(n p) f -> p n f", p=128)))
                        xh.append(xbuf)
                    if tt == 0:
                        nc.sync.dma_start(
                            out=wv_sb,
                            in_=r32(wv.ap().rearrange("(n p) f -> p n f", p=128)))
                    if tt % 4 == 0:
                        for h in range(HPC):
                            v_pre[(b, h)] = p2v.tile(
                                [128, 16, 128], BF16, tag="vsb", name=f"v{b}{h}")
                    # c-outer so compute chases the DMA stream chunk by chunk
                    ps = [p1ps.tile([128, 512], F32, tag="qkps", name=f"qk{fb}")
                          for fb in range(4)]
                    psv = [p1psv.tile([128, FLOC], F32, tag="vps", name=f"v{tb}")
                           for tb in range(4)]
                    for c in range(16):
                        half, cc = c // 8, c % 8
                        for fb in range(4):
                            nc.tensor.matmul(
                                ps[fb],
                                lhsT=wqk_sb[:, c, fb * 128:(fb + 1) * 128],
                                rhs=xh[half][:, cc, :],
                                start=(c == 0), stop=(c == 15),
                            )
                        for tb in range(4):
                            nc.tensor.matmul(
                                psv[tb],
                                lhsT=xh[half][:, cc, tb * 128:(tb + 1) * 128],
                                rhs=wv_sb[:, c, :],
                                start=(c == 0), stop=(c == 15),
                            )
                    for fb in range(4):
                        nc.vector.tensor_copy(
                            qk_res[:, fb, tt * 512:(tt + 1) * 512], ps[fb])
                    for tb in range(4):
                        for h in range(HPC):
                            nc.vector.tensor_copy(
                                v_pre[(b, h)][:, (tt % 4) * 4 + tb, :],
                                psv[tb][:, h * 128:(h + 1) * 128])

            # ============ phases 2+3: attention + per-batch a2a + proj ======
            with (
                tc.tile_pool(name="p4w", bufs=4) as p4w,
                tc.tile_pool(name="p4y", bufs=2) as p4y,
                tc.tile_pool(name="p4r", bufs=2) as p4r,
                tc.tile_pool(name="p4rb", bufs=1) as p4rb,
                tc.tile_pool(name="p4s", bufs=3) as p4s,
                tc.tile_pool(name="p4ps", bufs=2, space="PSUM") as p4ps,
            ):
                wp_tiles = []

                def load_wp_chunk(ch):
                    wt = p4w.tile([128, 16, 512], BF16, tag="wp", name=f"wp{ch}")
                    nc.sync.dma_start(
                        out=wt,
                        in_=wp[:, ch * 512:(ch + 1) * 512].rearrange(
                            "(n p) f -> p n f", p=128))
                    wp_tiles.append(wt)

                yts = []

                def recv_batch(b):
                    """post-a2a: load y/r for my 512 tokens, normalize y.
                    yts chunk index = h*8 + s  (w_proj row block g=2s+h)."""
                    rr = p4r.tile([16, 256], BF16, tag="rrecv", name=f"rr{b}")
                    src = y_t[b].ap().rearrange("(s r) t -> r s t", r=SROW)
                    for h in range(HPC):
                        nc.sync.dma_start(out=rr[h * 8:(h + 1) * 8, :],
                                          in_=src[256 + h])
                    ri = p4r.tile([16, 256], BF16, tag="rinv", name=f"ri{b}")
                    with nc.allow_low_precision(reason="softmax denom bf16"):
                        nc.vector.reciprocal(ri, rr)
                    nc.sync.dma_start(out=ri_dram[b].ap(), in_=ri)
                    rb = p4rb.tile([128, 16, 256], BF16, tag="rb", name=f"rb{b}")
                    nc.sync.dma_start(
                        out=rb, in_=ri_dram[b].ap().to_broadcast([128, 16, 256]))
                    yb = p4y.tile([128, 16, 256], BF16, tag="yt", name=f"yt{b}")
                    for h in range(HPC):
                        nc.sync.dma_start(
                            out=yb[:, h * 8:(h + 1) * 8, :],
                            in_=src[h * 128:(h + 1) * 128])
                    nc.vector.tensor_mul(yb, yb, rb)
                    yts.append(yb)

                with (
                    tc.tile_pool(name="p2p", bufs=3) as p2p,
                    tc.tile_pool(name="p2r", bufs=2) as p2r,
                    tc.tile_pool(name="p2y", bufs=2) as p2y,
                    tc.tile_pool(name="p2pss", bufs=2, space="PSUM") as p2pss,
                    tc.tile_pool(name="p2psy", bufs=1, space="PSUM") as p2psy,
                    tc.tile_pool(name="p2psr", bufs=1, space="PSUM") as p2psr,
                ):
                    nwp = [0]
                    for b in range(B):
                        for h in range(HPC):
                            v_sb = v_pre[(b, h)]
                            qf, kf = h, 2 + h
                            tok0 = b * T
                            for j in range(4):
                                if b == 1 and h == 0 and nwp[0] < 3:
                                    # stream w_proj chunks during b1 attention
                                    load_wp_chunk(nwp[0])
                                    nwp[0] += 1
                                nk = 4 * j + 4
                                y_ps = p2psy.tile([128, 512], F32, tag="yps")
                                r_ps = p2psr.tile([1, 512], F32, tag="rps")
                                r_accv = p2r.tile([128, 512], F32R, tag="rav")
                                r_accg = p2r.tile([128, 512], F32R, tag="rag")
                                qs = qk_res[:, qf,
                                            tok0 + j * 512: tok0 + (j + 1) * 512]
                                pe_rows = [c for c in range(nk) if c % 8 == 7]
                                nv = ng = 0
                                for t in range(nk // 2):
                                    s_ps = p2pss.tile([128, 2, 512], F32, tag="sps")
                                    for e in range(2):
                                        c = 2 * t + e
                                        diag = c - 4 * j
                                        nc.tensor.matmul(
                                            s_ps[:, e, :],
                                            lhsT=qk_res[:, kf,
                                                        tok0 + c * 128:
                                                        tok0 + (c + 1) * 128],
                                            rhs=qs,
                                            start=True, stop=(diag < 0),
                                        )
                                        if diag >= 0:
                                            nc.tensor.matmul(
                                                s_ps[:, e, :],
                                                lhsT=ident_sb,
                                                rhs=maskb_sb[:, diag, :],
                                                start=False, stop=True,
                                            )
                                    p_sb = p2p.tile([128, 2, 512], BF16, tag="p")
                                    nc.scalar.activation(
                                        p_sb, s_ps,
                                        mybir.ActivationFunctionType.Exp,
                                        scale=SCALE, bias=zeros_sb,
                                    )
                                    for e in range(2):
                                        c = 2 * t + e
                                        nc.tensor.matmul(
                                            y_ps,
                                            lhsT=v_sb[:, c, :],
                                            rhs=p_sb[:, e, :],
                                            start=(c == 0), stop=(c == nk - 1),
                                        )
                                        if c in pe_rows:
                                            nc.tensor.matmul(
                                                r_ps,
                                                lhsT=onesb_sb,
                                                rhs=p_sb[:, e, :],
                                                start=(c == pe_rows[0]),
                                                stop=False,
                                            )
                                        elif c % 2 == 0:
                                            if nv == 0:
                                                nc.vector.tensor_copy(
                                                    r_accv, p_sb[:, e, :])
                                            else:
                                                nc.vector.tensor_add(
                                                    r_accv, r_accv, p_sb[:, e, :])
                                            nv += 1
                                        else:
                                            if ng == 0:
                                                nc.gpsimd.tensor_copy(
                                                    r_accg, p_sb[:, e, :])
                                            else:
                                                nc.gpsimd.tensor_add(
                                                    r_accg, r_accg, p_sb[:, e, :])
                                            ng += 1
                                # fold the engine accumulators into r_ps
                                nc.tensor.matmul(
                                    r_ps, lhsT=ones_sb, rhs=r_accv,
                                    start=(len(pe_rows) == 0), stop=False,
                                )
                                nc.tensor.matmul(
                                    r_ps, lhsT=ones_sb, rhs=r_accg,
                                    start=False, stop=True,
                                )
                                # r rows (bf16) -> y_loc shard s=2j+e, row 256+h
                                r_sb = p2r.tile([1, 512], BF16, tag="rsb")
                                nc.vector.tensor_copy(r_sb, r_ps)
                                for e in range(2):
                                    s = 2 * j + e
                                    nc.sync.dma_start(
                                        out=y_loc[b][s * SROW + 256 + h:
                                                     s * SROW + 256 + h + 1, :],
                                        in_=r_sb[0:1, e * 256:(e + 1) * 256],
                                    )
                                y_sb = p2y.tile([128, 512], BF16, tag="ysb")
                                nc.vector.tensor_copy(y_sb, y_ps)
                                for e in range(2):
                                    s = 2 * j + e
                                    nc.sync.dma_start(
                                        out=y_loc[b][s * SROW + h * 128:
                                                     s * SROW + (h + 1) * 128, :],
                                        in_=y_sb[:, e * 256:(e + 1) * 256],
                                    )
                        # batch-b a2a; b=0's overlaps b=1 attention
                        a2a_op(y_loc[b], y_t[b])
                        recv_batch(b)
                    while nwp[0] < 4:
                        load_wp_chunk(nwp[0])
                        nwp[0] += 1
                    # projection: all of b0 (overlaps b1 a2a tail), then b1;
                    # tb groups interleaved so LDWEIGHTS pipelines with streams
                    for b in range(B):
                        for ch in range(4):
                            pps = [p4ps.tile([128, 512], F32, tag="ops",
                                             name=f"ops{tb}") for tb in range(2)]
                            for c in range(16):
                                yi = (c % 2) * 8 + c // 2   # chunk g=2s+h -> h*8+s
                                for tb in range(2):
                                    nc.tensor.matmul(
                                        pps[tb],
                                        lhsT=yts[b][:, yi, tb * 128:(tb + 1) * 128],
                                        rhs=wp_tiles[ch][:, c, :],
                                        start=(c == 0), stop=(c == 15),
                                    )
                            for tb in range(2):
                                st = p4s.tile([128, 512], F32, tag="ost")
                                nc.vector.tensor_copy(st, pps[tb])
                                nc.sync.dma_start(
                                    out=out[b * 256 + tb * 128:
                                            b * 256 + (tb + 1) * 128,
                                            ch * 512:(ch + 1) * 512],
                                    in_=st,
                                )
            p2v_ctx.__exit__(None, None, None)

    nc.compile()
    return nc


def make_in_maps(x, w_attn, w_proj):
    x = np.asarray(x, dtype=np.float32)
    w_attn = np.asarray(w_attn, dtype=np.float32)
    w_proj = np.asarray(w_proj, dtype=np.float32)
    xt = np.ascontiguousarray(x.reshape(NTOK, C).T)          # [C, NTOK]
    wp = np.ascontiguousarray(w_proj.astype(NPBF16))
    in_maps = []
    for i in range(NCORES):
        qcols = w_attn[:, FLOC * i: FLOC * (i + 1)]
        kcols = w_attn[:, C + FLOC * i: C + FLOC * (i + 1)]
        vcols = w_attn[:, 2 * C + FLOC * i: 2 * C + FLOC * (i + 1)]
        in_maps.append({
            "xt": xt,
            "wqk": np.ascontiguousarray(np.concatenate([qcols, kcols], axis=1)),
            "wv": np.ascontiguousarray(vcols),
            "wp": wp,
        })
    return in_maps


def kernel(x, w_attn, w_proj):
    global last_exec_time_ns
    if "nc" not in _cache:
        _cache["nc"] = build_nc()
    nc = _cache["nc"]
    in_maps = make_in_maps(x, w_attn, w_proj)
    res = run_bass_kernel_spmd(nc, in_maps, list(range(NCORES)))
    last_exec_time_ns = res.exec_time_ns
    return assemble([res.results[g]["out"] for g in range(NCORES)])


def assemble(outs):
    # core g's out rows: [0:256] = batch0 tokens [256g:256(g+1)],
    #                    [256:512] = batch1 tokens [256g:256(g+1)]
    full = np.empty((B, T, C), np.float32)
    for g in range(NCORES):
        for b in range(B):
            full[b, 256 * g: 256 * (g + 1), :] = outs[g][b * 256:(b + 1) * 256]
    return full


# revision 20
# speedup vs baseline: 1.1570x; 1.0015x over previous
"""Causal self-attention TP kernel for 8 trn2 NeuronCores.

Problem shapes (hardcoded): x [2, 2048, 2048] f32, w_attn [2048, 6144],
w_proj [2048, 2048], 16 heads, head_dim 128.

Sharding: tensor-parallel over heads — core i owns heads {2i, 2i+1} for BOTH
batches. Each core computes its local-head qkv + attention, producing
y_local^T [256 feat, 4096 tok] (unnormalized) plus per-token softmax row
sums r, shipped IN-BAND with y: one AllToAll per batch re-shards 258-row
shards [128 h0-feat | 128 h1-feat | r_h0 | r_h1] x 256 tok from
feature-split to token-split. The receiver normalizes y by 1/r per head
(normalization commutes with the projection within each head block) and
projects its 512 tokens against the full w_proj.

Precision: q/k/x/w_attn stay fp32r (full PE rate); p = exp(scores), v, y, r
(transport), w_proj are bf16; row sums accumulate in f32. The causal mask is
folded into the score PSUM accumulation as an identity x (-1e9 staircase)
matmul. Row sums: P tiles accumulate on DVE/GpSimd (alternating c, separate
f32 accumulators); every 8th tile goes through a per-tile ones-matmul on the
PE; two final ones-matmuls per q-tile fold the accumulators into PSUM.
"""

import numpy as np
import ml_dtypes

import concourse.bass as bass
import concourse.mybir as mybir
import concourse.tile as tile
from concourse import bacc
from concourse.bass_utils import run_bass_kernel_spmd

F32 = mybir.dt.float32
F32R = mybir.dt.float32r
BF16 = mybir.dt.bfloat16
NPBF16 = ml_dtypes.bfloat16

B, T, C = 2, 2048, 2048
H, D = 16, 128
NTOK = B * T                     # 4096 flat tokens (batch-major)
SCALE = 1.0 / float(np.sqrt(D))  # 0.08838834764831845
NCORES = 8
HPC = H // NCORES                # 2 heads per core
FLOC = HPC * D                   # 256 local v features
QK = 512                         # q+k local features (2 heads x 128 x 2)
NEG = -1.0e9                     # causal mask bias (exp(SCALE*NEG) == 0)
SROW = 129                       # a2a shard rows: 128 feat + 1 r row

last_exec_time_ns = None
_cache = {}


def r32(ap):
    return ap.bitcast(F32R)


def _maskbias_np():
    # mb[m, kk, qq] = 0 iff kk <= qq - 128*m else NEG (diagonal tile offset m)
    m = np.arange(4)[:, None, None]
    kk = np.arange(128)[None, :, None]
    qq = np.arange(512)[None, None, :]
    return np.where(kk <= qq - 128 * m, 0.0, NEG).astype(np.float32)


def build_nc(no_collective=False, reps=1):
    nc = bacc.Bacc("TRN2", target_bir_lowering=False, debug=False,
                   num_devices=1 if no_collective else NCORES)

    xt = nc.dram_tensor("xt", [C, NTOK], F32, kind="ExternalInput")
    wqk = nc.dram_tensor("wqk", [C, QK], F32, kind="ExternalInput")
    wv = nc.dram_tensor("wv", [C, FLOC], F32, kind="ExternalInput")
    wp = nc.dram_tensor("wp", [C, C], BF16, kind="ExternalInput")
    out = nc.dram_tensor("out", [512, C], F32, kind="ExternalOutput")

    # per-(batch, head) a2a buffers: 8 shards x [129 rows x 256 tok] bf16
    y_loc = {(b, h): nc.dram_tensor(f"y_loc{b}{h}", [8 * SROW, 256], BF16)
             for b in range(B) for h in range(HPC)}
    y_t = {(b, h): nc.dram_tensor(f"y_t{b}{h}", [8 * SROW, 256], BF16)
           for b in range(B) for h in range(HPC)}
    ri_dram = [nc.dram_tensor(f"ri{b}", [1, 16, 256], BF16) for b in range(B)]
    warm = [nc.dram_tensor(f"warm{k}", [8, 2], F32) for k in range(4)]
    warm_t = [nc.dram_tensor(f"warm_t{k}", [8, 2], F32) for k in range(4)]

    maskb_dr = nc.inline_tensor(_maskbias_np(), "maskb")      # [4, 128, 512]
    ident_dr = nc.inline_tensor(np.eye(128, dtype=np.float32), "ident")
    ones_dr = nc.inline_tensor(np.ones((128, 1), np.float32), "ones_c")
    onesb_dr = nc.inline_tensor(np.ones((128, 1), NPBF16), "onesb_c")
    zeros_dr = nc.inline_tensor(np.zeros((128, 1), np.float32), "zeros_c")

    def a2a_op(src, dst):
        if no_collective:
            nc.gpsimd.dma_start(out=dst[:, :], in_=src[:, :])
        else:
            nc.gpsimd.collective_compute(
                "AllToAll", mybir.AluOpType.bypass,
                replica_groups=[list(range(NCORES))],
                ins=[src[:, :]], outs=[dst[:, :]],
            )

    with tile.TileContext(nc) as tc:
      for _rep in range(reps):
        # ---- persistent (qkv outputs + constants) ----
        with tc.tile_pool(name="persist", bufs=1) as persist:
            # q^T,k^T for 2 heads, all tokens: chunk f = {q_h0, q_h1, k_h0, k_h1}
            qk_res = persist.tile([128, 4, NTOK], F32R)
            ones_sb = persist.tile([128, 1], F32R)
            onesb_sb = persist.tile([128, 1], BF16)
            zeros_sb = persist.tile([128, 1], F32)
            nc.gpsimd.dma_start(out=zeros_sb, in_=zeros_dr.ap())
            maskb_sb = persist.tile([128, 4, 512], F32R)
            nc.gpsimd.dma_start(out=maskb_sb,
                                in_=r32(maskb_dr.ap().rearrange("m p q -> p m q")))
            ident_sb = persist.tile([128, 128], F32R)
            nc.gpsimd.dma_start(out=ident_sb, in_=r32(ident_dr.ap()))
            scr = persist.tile([128, 1], F32)
            # warm the ACT exp table set (~2.7us) before attention needs it
            nc.scalar.activation(scr, zeros_sb,
                                 mybir.ActivationFunctionType.Exp, bias=zeros_sb)
            nc.gpsimd.dma_start(out=ones_sb, in_=r32(ones_dr.ap()))
            nc.gpsimd.dma_start(out=onesb_sb, in_=onesb_dr.ap())
            # warm the collective stream with a dummy a2a (overlaps qkv);
            # more warmers are trickled through phase 1 to keep it hot
            a2a_op(warm[0], warm_t[0])

            # v for all (batch, head), bf16, SBUF-resident: [tok128, ktile, feat]
            p2v_ctx = tc.tile_pool(name="p2v", bufs=4)
            p2v = p2v_ctx.__enter__()
            v_pre = {}

            # ================= phase 1: qkv =================
            with (
                tc.tile_pool(name="p1w", bufs=1) as p1w,
                tc.tile_pool(name="p1x", bufs=3) as p1x,
                tc.tile_pool(name="p1ps", bufs=4, space="PSUM") as p1ps,
                tc.tile_pool(name="p1psv", bufs=4, space="PSUM") as p1psv,
            ):
                wqk_sb = p1w.tile([128, 16, QK], F32R)
                wv_sb = p1w.tile([128, 16, FLOC], F32R)

                for tt in range(8):          # 512-token tiles over 4096 tokens
                    b = tt // 4
                    xh = []
                    for half in range(2):
                        xbuf = p1x.tile([128, 8, 512], F32R, tag="xh")
                        c0 = half * 8
                        if tt == 0:
                            # chase: per-chunk loads, weights on sync queue,
                            # x on the (idle) scalar queue, paired per chunk
                            for cc in range(8):
                                nc.sync.dma_start(
                                    out=wqk_sb[:, c0 + cc, :],
                                    in_=r32(wqk[(c0 + cc) * 128:(c0 + cc + 1) * 128, :]))
                                nc.scalar.dma_start(
                                    out=xbuf[:, cc, :],
                                    in_=r32(xt[(c0 + cc) * 128:(c0 + cc + 1) * 128,
                                               tt * 512:(tt + 1) * 512]))
                        else:
                            nc.sync.dma_start(
                                out=xbuf,
                                in_=r32(xt[c0 * 128:(c0 + 8) * 128,
                                           tt * 512:(tt + 1) * 512].rearrange(
                                               "(n p) f -> p n f", p=128)))
                        xh.append(xbuf)
                    if tt == 0:
                        nc.sync.dma_start(
                            out=wv_sb,
                            in_=r32(wv.ap().rearrange("(n p) f -> p n f", p=128)))
                    if tt % 4 == 0:
                        for h in range(HPC):
                            v_pre[(b, h)] = p2v.tile(
                                [128, 16, 128], BF16, tag="vsb", name=f"v{b}{h}")
                    # c-outer so compute chases the DMA stream chunk by chunk
                    ps = [p1ps.tile([128, 512], F32, tag="qkps", name=f"qk{fb}")
                          for fb in range(4)]
                    psv = [p1psv.tile([128, FLOC], F32, tag="vps", name=f"v{tb}")
                           for tb in range(4)]
                    for c in range(16):
                        half, cc = c // 8, c % 8
                        for fb in range(4):
                            nc.tensor.matmul(
                                ps[fb],
                                lhsT=wqk_sb[:, c, fb * 128:(fb + 1) * 128],
                                rhs=xh[half][:, cc, :],
                                start=(c == 0), stop=(c == 15),
                            )
                        for tb in range(4):
                            nc.tensor.matmul(
                                psv[tb],
                                lhsT=xh[half][:, cc, tb * 128:(tb + 1) * 128],
                                rhs=wv_sb[:, c, :],
                                start=(c == 0), stop=(c == 15),
                            )
                    for fb in range(4):
                        nc.vector.tensor_copy(
                            qk_res[:, fb, tt * 512:(tt + 1) * 512], ps[fb])
                    for tb in range(4):
                        for h in range(HPC):
                            nc.vector.tensor_copy(
                                v_pre[(b, h)][:, (tt % 4) * 4 + tb, :],
                                psv[tb][:, h * 128:(h + 1) * 128])
                    if tt in (2, 4, 6):
                        # keep the collective stream warm; the dummy a2a is
                        # tied to this tile's qk output so it fires ~now
                        k = tt // 2
                        nc.gpsimd.dma_start(
                            out=warm[k].ap(),
                            in_=qk_res[0:8, 0, tt * 512:tt * 512 + 2].bitcast(F32))
                        a2a_op(warm[k], warm_t[k])

            # ============ phases 2+3: attention + per-batch a2a + proj ======
            with (
                tc.tile_pool(name="p4w", bufs=4) as p4w,
                tc.tile_pool(name="p4y", bufs=2) as p4y,
                tc.tile_pool(name="p4r", bufs=2) as p4r,
                tc.tile_pool(name="p4rb", bufs=1) as p4rb,
                tc.tile_pool(name="p4s", bufs=3) as p4s,
                tc.tile_pool(name="p4ps", bufs=2, space="PSUM") as p4ps,
            ):
                wp_tiles = []

                def load_wp_chunk(ch):
                    wt = p4w.tile([128, 16, 512], BF16, tag="wp", name=f"wp{ch}")
                    nc.sync.dma_start(
                        out=wt,
                        in_=wp[:, ch * 512:(ch + 1) * 512].rearrange(
                            "(n p) f -> p n f", p=128))
                    wp_tiles.append(wt)

                yts = []

                def recv_batch(b):
                    """post-a2a: load y/r for my 512 tokens, normalize y.
                    yts chunk index = h*8 + s  (w_proj row block g=2s+h)."""
                    rr = p4r.tile([16, 256], BF16, tag="rrecv", name=f"rr{b}")
                    yb = p4y.tile([128, 16, 256], BF16, tag="yt", name=f"yt{b}")
                    for h in range(HPC):
                        src = y_t[(b, h)].ap().rearrange(
                            "(s r) t -> r s t", r=SROW)
                        nc.sync.dma_start(out=rr[h * 8:(h + 1) * 8, :],
                                          in_=src[128])
                        nc.sync.dma_start(out=yb[:, h * 8:(h + 1) * 8, :],
                                          in_=src[0:128])
                    ri = p4r.tile([16, 256], BF16, tag="rinv", name=f"ri{b}")
                    with nc.allow_low_precision(reason="softmax denom bf16"):
                        nc.vector.reciprocal(ri, rr)
                    nc.sync.dma_start(out=ri_dram[b].ap(), in_=ri)
                    rb = p4rb.tile([128, 16, 256], BF16, tag="rb", name=f"rb{b}")
                    nc.sync.dma_start(
                        out=rb, in_=ri_dram[b].ap().to_broadcast([128, 16, 256]))
                    nc.vector.tensor_mul(yb, yb, rb)
                    yts.append(yb)

                with (
                    tc.tile_pool(name="p2p", bufs=3) as p2p,
                    tc.tile_pool(name="p2r", bufs=2) as p2r,
                    tc.tile_pool(name="p2y", bufs=2) as p2y,
                    tc.tile_pool(name="p2pss", bufs=2, space="PSUM") as p2pss,
                    tc.tile_pool(name="p2psy", bufs=1, space="PSUM") as p2psy,
                    tc.tile_pool(name="p2psr", bufs=1, space="PSUM") as p2psr,
                ):
                    nwp = [0]
                    for b in range(B):
                        for h in range(HPC):
                            v_sb = v_pre[(b, h)]
                            qf, kf = h, 2 + h
                            tok0 = b * T
                            for j in range(4):
                                if b == 1 and nwp[0] < 2 * h + 2:
                                    # stream w_proj chunks during b1 attention
                                    load_wp_chunk(nwp[0])
                                    nwp[0] += 1
                                nk = 4 * j + 4
                                y_ps = p2psy.tile([128, 512], F32, tag="yps")
                                r_ps = p2psr.tile([1, 512], F32, tag="rps")
                                r_accv = p2r.tile([128, 512], F32R, tag="rav")
                                r_accg = p2r.tile([128, 512], F32R, tag="rag")
                                qs = qk_res[:, qf,
                                            tok0 + j * 512: tok0 + (j + 1) * 512]
                                pe_rows = [c for c in range(nk) if c % 8 == 7]
                                nv = ng = 0

                                def issue_pair(t):
                                    s_ps = p2pss.tile([128, 2, 512], F32,
                                                      tag="sps")
                                    for e in range(2):
                                        c = 2 * t + e
                                        diag = c - 4 * j
                                        nc.tensor.matmul(
                                            s_ps[:, e, :],
                                            lhsT=qk_res[:, kf,
                                                        tok0 + c * 128:
                                                        tok0 + (c + 1) * 128],
                                            rhs=qs,
                                            start=True, stop=(diag < 0),
                                        )
                                        if diag >= 0:
                                            nc.tensor.matmul(
                                                s_ps[:, e, :],
                                                lhsT=ident_sb,
                                                rhs=maskb_sb[:, diag, :],
                                                start=False, stop=True,
                                            )
                                    p_sb = p2p.tile([128, 2, 512], BF16,
                                                    tag="p")
                                    nc.scalar.activation(
                                        p_sb, s_ps,
                                        mybir.ActivationFunctionType.Exp,
                                        scale=SCALE, bias=zeros_sb,
                                    )
                                    return p_sb

                                # software pipeline: pair t+1's scores issue
                                # before pair t's PV so PE hides exp latency
                                p_cur = issue_pair(0)
                                for t in range(nk // 2):
                                    p_nxt = (issue_pair(t + 1)
                                             if t + 1 < nk // 2 else None)
                                    for e in range(2):
                                        c = 2 * t + e
                                        nc.tensor.matmul(
                                            y_ps,
                                            lhsT=v_sb[:, c, :],
                                            rhs=p_cur[:, e, :],
                                            start=(c == 0), stop=(c == nk - 1),
                                        )
                                        if c in pe_rows:
                                            nc.tensor.matmul(
                                                r_ps,
                                                lhsT=onesb_sb,
                                                rhs=p_cur[:, e, :],
                                                start=(c == pe_rows[0]),
                                                stop=False,
                                            )
                                        elif c % 2 == 0:
                                            if nv == 0:
                                                nc.vector.tensor_copy(
                                                    r_accv, p_cur[:, e, :])
                                            else:
                                                nc.vector.tensor_add(
                                                    r_accv, r_accv,
                                                    p_cur[:, e, :])
                                            nv += 1
                                        else:
                                            if ng == 0:
                                                nc.gpsimd.tensor_copy(
                                                    r_accg, p_cur[:, e, :])
                                            else:
                                                nc.gpsimd.tensor_add(
                                                    r_accg, r_accg,
                                                    p_cur[:, e, :])
                                            ng += 1
                                    p_cur = p_nxt
                                # fold the engine accumulators into r_ps
                                nc.tensor.matmul(
                                    r_ps, lhsT=ones_sb, rhs=r_accv,
                                    start=(len(pe_rows) == 0), stop=False,
                                )
                                nc.tensor.matmul(
                                    r_ps, lhsT=ones_sb, rhs=r_accg,
                                    start=False, stop=True,
                                )
                                # r row (bf16) -> y_loc[(b,h)] shard row 128
                                r_sb = p2r.tile([1, 512], BF16, tag="rsb")
                                nc.vector.tensor_copy(r_sb, r_ps)
                                for e in range(2):
                                    s = 2 * j + e
                                    nc.sync.dma_start(
                                        out=y_loc[(b, h)][s * SROW + 128:
                                                          s * SROW + 129, :],
                                        in_=r_sb[0:1, e * 256:(e + 1) * 256],
                                    )
                                y_sb = p2y.tile([128, 512], BF16, tag="ysb")
                                nc.vector.tensor_copy(y_sb, y_ps)
                                for e in range(2):
                                    s = 2 * j + e
                                    nc.sync.dma_start(
                                        out=y_loc[(b, h)][s * SROW:
                                                          s * SROW + 128, :],
                                        in_=y_sb[:, e * 256:(e + 1) * 256],
                                    )
                            # per-head a2a: h0's fires at 25%/75% of attention
                            a2a_op(y_loc[(b, h)], y_t[(b, h)])
                        recv_batch(b)
                    while nwp[0] < 4:
                        load_wp_chunk(nwp[0])
                        nwp[0] += 1
                    # projection: all of b0 (overlaps b1 a2a tail), then b1;
                    # tb groups interleaved so LDWEIGHTS pipelines with streams
                    for b in range(B):
                        for ch in range(4):
                            pps = [p4ps.tile([128, 512], F32, tag="ops",
                                             name=f"ops{tb}") for tb in range(2)]
                            for c in range(16):
                                yi = (c % 2) * 8 + c // 2   # chunk g=2s+h -> h*8+s
                                for tb in range(2):
                                    nc.tensor.matmul(
                                        pps[tb],
                                        lhsT=yts[b][:, yi, tb * 128:(tb + 1) * 128],
                                        rhs=wp_tiles[ch][:, c, :],
                                        start=(c == 0), stop=(c == 15),
                                    )
                            for tb in range(2):
                                st = p4s.tile([128, 512], F32, tag="ost")
                                nc.vector.tensor_copy(st, pps[tb])
                                nc.sync.dma_start(
                                    out=out[b * 256 + tb * 128:
                                            b * 256 + (tb + 1) * 128,
                                            ch * 512:(ch + 1) * 512],
                                    in_=st,
                                )
            p2v_ctx.__exit__(None, None, None)

    nc.compile()
    return nc


def make_in_maps(x, w_attn, w_proj):
    x = np.asarray(x, dtype=np.float32)
    w_attn = np.asarray(w_attn, dtype=np.float32)
    w_proj = np.asarray(w_proj, dtype=np.float32)
    xt = np.ascontiguousarray(x.reshape(NTOK, C).T)          # [C, NTOK]
    wp = np.ascontiguousarray(w_proj.astype(NPBF16))
    in_maps = []
    for i in range(NCORES):
        qcols = w_attn[:, FLOC * i: FLOC * (i + 1)]
        kcols = w_attn[:, C + FLOC * i: C + FLOC * (i + 1)]
        vcols = w_attn[:, 2 * C + FLOC * i: 2 * C + FLOC * (i + 1)]
        in_maps.append({
            "xt": xt,
            "wqk": np.ascontiguousarray(np.concatenate([qcols, kcols], axis=1)),
            "wv": np.ascontiguousarray(vcols),
            "wp": wp,
        })
    return in_maps


def kernel(x, w_attn, w_proj):
    global last_exec_time_ns
    if "nc" not in _cache:
        _cache["nc"] = build_nc()
    nc = _cache["nc"]
    in_maps = make_in_maps(x, w_attn, w_proj)
    res = run_bass_kernel_spmd(nc, in_maps, list(range(NCORES)))
    last_exec_time_ns = res.exec_time_ns
    return assemble([res.results[g]["out"] for g in range(NCORES)])


def assemble(outs):
    # core g's out rows: [0:256] = batch0 tokens [256g:256(g+1)],
    #                    [256:512] = batch1 tokens [256g:256(g+1)]
    full = np.empty((B, T, C), np.float32)
    for g in range(NCORES):
        for b in range(B):
            full[b, 256 * g: 256 * (g + 1), :] = outs[g][b * 256:(b + 1) * 256]
    return full
